# revision 36
# baseline (speedup 1.0000x reference)
"""DeltaNet forward on 8 TRN2 NeuronCores (Bass/Tile kernel).

Contract: kernel(**inputs) takes the FULL inputs of reference.setup_inputs()
and returns the FULL [B, L, D] output.

Sharding (hardcoded): 16 (batch, head) shards on 8 cores — core c handles
batch c//2 and heads {2*(c%2), 2*(c%2)+1}. Projection weights are sliced
per-core on the host; each core computes conv+silu for its batch, projects
q/k/v/beta/g for its two heads, runs the chunked delta rule (chunk=128,
(I+A)^-1 via Newton iteration — exact since A is strictly triangular and
nilpotent), applies the fused RMSNorm-swish gate and its slice of the output
projection. The host sums the two partial outputs per batch (tensor-parallel
output projection over heads).

Math notes:
 - q is NOT l2-normalized on device: each output row o[t] is linear in q[t],
   so the 1/||q[t]|| factor commutes through to the rms-norm, which is
   computed as o / sqrt(mean(o^2) + eps * ||q||^2)  — exact identity.
 - norm_w is folded into Wo on the host (diag scale on the contracted dim).
 - All matmuls use fp16 operands with fp32 PSUM accumulation.
"""

import os
import sys
import numpy as np

for _p in ("/opt/trn_rl_repo", "/root/.axon_site/_ro/trn_rl_repo"):
    if os.path.isdir(_p) and _p not in sys.path:
        sys.path.append(_p)

# ---- problem constants (hardcoded from the spec) ----
B, L, D = 4, 4096, 1024
H = 4
DK, DV = 512, 1024
DQH, DVH = DK // H, DV // H          # 128, 256
KC = 4                                # conv width
NORM_EPS = 1e-5
C = 128                               # delta-rule chunk (math-equiv to any)
SC = 512                              # superchunk (conv/dma granularity)
NEWTON_ITERS = 3                      # error = A^16 ~ 0 numerically (verified)
CONV_PE_KT = 3                        # D-tiles computed on PE (diag matmuls)

N_CORES = 8
TRACE = False                         # test.py flips this for profiling
TRACE_KW = {}
SIM_COMPAT = False                    # decompose Silu (CoreSim lacks it)

_BUILT = None


def _build(nc_L=L):
    import concourse.bass as bass
    import concourse.tile as tile
    import concourse.mybir as mybir
    from contextlib import ExitStack

    F32 = mybir.dt.float32
    F16 = mybir.dt.float16
    I32 = mybir.dt.int32
    AF = mybir.ActivationFunctionType
    OP = mybir.AluOpType

    n_sc = nc_L // SC
    n_cc_per_sc = SC // C

    nc = bass.Bass("TRN2", target_bir_lowering=False, debug=False,
                   num_devices=N_CORES)

    KT = D // 128  # 8 K-tiles

    xT = nc.dram_tensor("xT", [D, nc_L + KC - 1], F16, kind="ExternalInput").ap()
    wqk = nc.dram_tensor("wqk", [D, 512], F16, kind="ExternalInput").ap()
    wv = nc.dram_tensor("wv", [D, 512], F16, kind="ExternalInput").ap()
    wg = nc.dram_tensor("wg", [D, 512], F16, kind="ExternalInput").ap()
    wb = nc.dram_tensor("wb", [D, 2], F16, kind="ExternalInput").ap()
    wo = nc.dram_tensor("wo", [512, D], F16, kind="ExternalInput").ap()
    cw = nc.dram_tensor("cw", [D, KC], F32, kind="ExternalInput").ap()
    cwdiag = nc.dram_tensor("cwdiag", [KT * KC * 128, 128], F16,
                            kind="ExternalInput").ap()
    ident = nc.dram_tensor("ident", [128, 128], F16, kind="ExternalInput").ap()
    ident4 = nc.dram_tensor("ident4", [128, 512], F16, kind="ExternalInput").ap()
    mtril = nc.dram_tensor("mtril", [128, 128], F32, kind="ExternalInput").ap()
    mtriu4 = nc.dram_tensor("mtriu4", [128, 512], F32, kind="ExternalInput").ap()
    out_d = nc.dram_tensor("out", [nc_L, D], F32, kind="ExternalOutput").ap()

    with tile.TileContext(nc) as tc, ExitStack() as ctx:
        wpool = ctx.enter_context(tc.tile_pool(name="w", bufs=1))
        xpool = ctx.enter_context(tc.tile_pool(name="x", bufs=2))
        hpool = ctx.enter_context(tc.tile_pool(name="h", bufs=2))
        cpool = ctx.enter_context(tc.tile_pool(name="cacc", bufs=2))
        spool = ctx.enter_context(tc.tile_pool(name="s", bufs=2))
        kpool = ctx.enter_context(tc.tile_pool(name="k", bufs=4))
        npool = ctx.enter_context(tc.tile_pool(name="n", bufs=6))
        opool = ctx.enter_context(tc.tile_pool(name="o", bufs=2))
        ps512 = ctx.enter_context(tc.tile_pool(name="ps512", bufs=2, space="PSUM"))
        psop = ctx.enter_context(tc.tile_pool(name="psop", bufs=2, space="PSUM"))
        pscc = ctx.enter_context(tc.tile_pool(name="pscc", bufs=2, space="PSUM"))
        psu = ctx.enter_context(tc.tile_pool(name="psu", bufs=2, space="PSUM"))

        # prefetch the first x superchunk before the bulk weight loads
        x_first = xpool.tile([128, KT, SC + KC - 1], F16, tag="x")
        nc.sync.dma_start(
            x_first[:],
            xT[:, 0: SC + KC - 1].rearrange("(k p) l -> p k l", p=128))

        # ---- constants / weights (resident) ----
        wqk_s = wpool.tile([128, KT, 512], F16, tag="wqk")
        nc.sync.dma_start(wqk_s[:], wqk.rearrange("(k p) c -> p k c", p=128))
        wv_s = wpool.tile([128, KT, 512], F16, tag="wv")
        nc.sync.dma_start(wv_s[:], wv.rearrange("(k p) c -> p k c", p=128))
        wg_s = wpool.tile([128, KT, 512], F16, tag="wg")
        nc.sync.dma_start(wg_s[:], wg.rearrange("(k p) c -> p k c", p=128))
        wb_s = wpool.tile([128, KT, 2], F16, tag="wb")
        nc.sync.dma_start(wb_s[:], wb.rearrange("(k p) c -> p k c", p=128))
        wo_s = wpool.tile([128, 4, D], F16, tag="wo")
        nc.sync.dma_start(wo_s[:], wo.rearrange("(j p) c -> p j c", p=128))
        cw_s = wpool.tile([128, KT, KC], F32, tag="cw")
        nc.sync.dma_start(cw_s[:], cw.rearrange("(k p) c -> p k c", p=128))
        cwd_s = wpool.tile([128, CONV_PE_KT * KC, 128], F16, tag="cwd")
        nc.sync.dma_start(cwd_s[:],
                          cwdiag[0:CONV_PE_KT * KC * 128, :].rearrange(
                              "(t p) c -> p t c", p=128))
        id_s = wpool.tile([128, 128], F16, tag="id")
        nc.sync.dma_start(id_s[:], ident)
        id4_s = wpool.tile([128, 512], F16, tag="id4")
        nc.sync.dma_start(id4_s[:], ident4)
        mtril_s = wpool.tile([128, 128], F32, tag="mtril")
        nc.sync.dma_start(mtril_s[:], mtril)
        mtriu4_s = wpool.tile([128, 512], F32, tag="mtriu4")
        nc.sync.dma_start(mtriu4_s[:], mtriu4)

        # ---- per-head state (fp16 working copy + fp32 master) ----
        S32 = []
        S16 = []
        for hh in range(2):
            s32 = spool.tile([128, DVH], F32, tag=f"S32_{hh}")
            s16 = spool.tile([128, DVH], F16, tag=f"S16_{hh}")
            nc.gpsimd.memset(s32[:], 0.0)
            nc.gpsimd.memset(s16[:], 0.0)
            S32.append(s32)
            S16.append(s16)

        def rsqrt_dve(z_ap, n, tagp):
            """y ~ 1/sqrt(z) elementwise on [128, n] f32, DVE only
            (fast-inverse-sqrt seed + 2 Newton iterations)."""
            zi = z_ap.bitcast(I32)
            t = kpool.tile([128, n], I32, tag=f"{tagp}_i")
            nc.vector.tensor_scalar(t[:], zi, 1, None, OP.arith_shift_right)
            nc.vector.tensor_scalar(t[:], t[:], -1, 0x5f3759df, OP.mult, OP.add)
            y = t[:].bitcast(F32)
            znh = kpool.tile([128, n], F32, tag=f"{tagp}_z")
            nc.vector.tensor_scalar_mul(znh[:], z_ap, -0.5)
            for _ in range(2):
                a = kpool.tile([128, n], F32, tag=f"{tagp}_a")
                nc.vector.tensor_mul(a[:], y, y)
                nc.vector.tensor_mul(a[:], a[:], znh[:])
                nc.vector.tensor_scalar_add(a[:], a[:], 1.5)
                y2 = kpool.tile([128, n], F32, tag=f"{tagp}_y")
                nc.vector.tensor_mul(y2[:], y, a[:])
                y = y2[:]
            return y

        for sc in range(n_sc):
            # ---- load x superchunk (host pre-padded by KC-1 zeros) ----
            if sc == 0:
                x_sb = x_first
            else:
                x_sb = xpool.tile([128, KT, SC + KC - 1], F16, tag="x")
                nc.sync.dma_start(
                    x_sb[:],
                    xT[:, sc * SC: sc * SC + SC + KC - 1].rearrange(
                        "(k p) l -> p k l", p=128),
                )
            # ---- causal depthwise conv + silu ----
            h_sb = hpool.tile([128, KT, SC], F16, tag="h")
            for k in range(KT):
                if k < CONV_PE_KT:
                    cv_ps = ps512.tile([128, SC], F32, tag="mm512")
                    for i in range(KC):
                        nc.tensor.matmul(cv_ps[:], cwd_s[:, k * KC + i, :],
                                         x_sb[:, k, i:i + SC],
                                         start=(i == 0), stop=(i == KC - 1))
                    src = cv_ps
                else:
                    acc = cpool.tile([128, SC], F32, tag="cacc")
                    nc.vector.tensor_scalar_mul(acc[:], x_sb[:, k, 0:SC],
                                                cw_s[:, k, 0:1])
                    for i in (1, 2):
                        nc.vector.scalar_tensor_tensor(
                            acc[:], x_sb[:, k, i:i + SC], cw_s[:, k, i:i + 1],
                            acc[:], OP.mult, OP.add)
                    acc2 = cpool.tile([128, SC], F32, tag="cacc2")
                    nc.vector.scalar_tensor_tensor(
                        acc2[:], x_sb[:, k, 3:3 + SC], cw_s[:, k, 3:4],
                        acc[:], OP.mult, OP.add)
                    src = acc2
                if SIM_COMPAT:
                    sgm = cpool.tile([128, SC], F32, tag="csig")
                    nc.scalar.activation(sgm[:], src[:], AF.Sigmoid)
                    nc.vector.tensor_mul(h_sb[:, k, :], src[:], sgm[:])
                else:
                    nc.scalar.activation(h_sb[:, k, :], src[:], AF.Silu)

            # ---- phase 1 per chunk: projections + squares ----
            sqk_sc = kpool.tile([128, 8], F32, tag="sqk_sc")
            sqq_sc = kpool.tile([128, 8], F32, tag="sqq_sc")
            ms_sc = kpool.tile([128, 8], F32, tag="ms_sc")
            z_sc = kpool.tile([128, 8], F32, tag="z_sc")
            chunk_data = []
            for lc in range(n_cc_per_sc):
                tok = bass.ts(lc, C)

                def hT(k, tok=tok):
                    return h_sb[:, k, tok]

                Q = {}
                qk_ps = ps512.tile([128, 512], F32, tag="mm512")
                for k in range(KT):
                    nc.tensor.matmul(qk_ps[:], hT(k), wqk_s[:, k, :],
                                     start=(k == 0), stop=(k == KT - 1))
                qk16 = kpool.tile([128, 512], F16, tag="qk16", bufs=5)
                nc.scalar.copy(qk16[:], qk_ps[:])
                Q["qk16"] = qk16

                b_ps = pscc.tile([128, 2], F32, tag="cc")
                for k in range(KT):
                    nc.tensor.matmul(b_ps[:], hT(k), wb_s[:, k, :],
                                     start=(k == 0), stop=(k == KT - 1))
                bth = kpool.tile([128, 2], F32, tag="bth")
                nc.scalar.activation(bth[:], b_ps[:], AF.Tanh, scale=0.5)
                beta = kpool.tile([128, 2], F32, tag="beta")
                nc.vector.tensor_scalar(beta[:], bth[:], 0.5, 0.5,
                                        OP.mult, OP.add)
                bneg = kpool.tile([128, 2], F32, tag="bneg")
                nc.vector.tensor_scalar(bneg[:], bth[:], -0.5, -0.5,
                                        OP.mult, OP.add)
                Q["beta"], Q["bneg"] = beta, bneg

                v_ps = ps512.tile([128, 512], F32, tag="mm512")
                for k in range(KT):
                    nc.tensor.matmul(v_ps[:], hT(k), wv_s[:, k, :],
                                     start=(k == 0), stop=(k == KT - 1))
                bv16 = kpool.tile([128, 512], F16, tag="bv", bufs=5)
                for hh in range(2):
                    nc.scalar.mul(bv16[:, hh * DVH:(hh + 1) * DVH],
                                  v_ps[:, hh * DVH:(hh + 1) * DVH],
                                  beta[:, hh:hh + 1])
                Q["bv"] = bv16

                g_ps = ps512.tile([128, 512], F32, tag="mm512")
                for k in range(KT):
                    nc.tensor.matmul(g_ps[:], hT(k), wg_s[:, k, :],
                                     start=(k == 0), stop=(k == KT - 1))
                sg16 = kpool.tile([128, 512], F16, tag="sg", bufs=5)
                if SIM_COMPAT:
                    gsg = kpool.tile([128, 512], F32, tag="gsig")
                    nc.scalar.activation(gsg[:], g_ps[:], AF.Sigmoid)
                    nc.vector.tensor_mul(sg16[:], g_ps[:], gsg[:])
                else:
                    nc.scalar.activation(sg16[:], g_ps[:], AF.Silu)
                Q["sg"] = sg16

                scr16 = kpool.tile([128, 128], F16, tag="scr16")
                for hh in range(2):
                    idx = lc * 2 + hh
                    nc.scalar.activation(scr16[:], qk16[:, bass.ts(hh, 128)],
                                         AF.Square,
                                         accum_out=sqq_sc[:, idx:idx + 1])
                    nc.scalar.activation(scr16[:],
                                         qk16[:, 256 + hh * 128:256 + (hh + 1) * 128],
                                         AF.Square,
                                         accum_out=sqk_sc[:, idx:idx + 1])
                chunk_data.append(Q)

            # ---- k-norm scales for the whole superchunk ----
            rs_all = rsqrt_dve(sqk_sc[:], 8, "rsk")

            # ---- phase 2a: k-scale, transposes, A — grouped across chunks
            # (keeps PE in one mode per group: fewer transpose<->matmul drains)
            for lc in range(n_cc_per_sc):
                Q = chunk_data[lc]
                qk16, bneg = Q["qk16"], Q["bneg"]
                kn16 = kpool.tile([128, 256], F16, tag="kn", bufs=8)
                kbn16 = kpool.tile([128, 256], F16, tag="kbn", bufs=8)
                for hh in range(2):
                    idx = lc * 2 + hh
                    ksl = slice(256 + hh * 128, 256 + (hh + 1) * 128)
                    hsl = bass.ts(hh, 128)
                    nc.scalar.mul(kn16[:, hsl], qk16[:, ksl],
                                  rs_all[:, idx:idx + 1])
                    nc.scalar.mul(kbn16[:, hsl], kn16[:, hsl],
                                  bneg[:, hh:hh + 1])
                Q["kn"], Q["kbn"] = kn16, kbn16
            for lp in range(n_cc_per_sc // 2):
                QA, QB = chunk_data[2 * lp], chunk_data[2 * lp + 1]
                # transposes: 2 chunks x 2 heads into one [128,512] psum bank
                kT_ps = pscc.tile([128, 512], F16, tag="cc")
                qT_ps = pscc.tile([128, 512], F16, tag="cc")
                for ci, Q in ((0, QA), (1, QB)):
                    for hh in range(2):
                        dsl = bass.ts(ci * 2 + hh, 128)
                        hsl = bass.ts(hh, 128)
                        nc.tensor.transpose(kT_ps[:, dsl], Q["kn"][:, hsl],
                                            id_s[:])
                        nc.tensor.transpose(qT_ps[:, dsl], Q["qk16"][:, hsl],
                                            id_s[:])
                kT16 = kpool.tile([128, 512], F16, tag="kT", bufs=4)
                nc.vector.tensor_copy(kT16[:], kT_ps[:])
                qT16 = kpool.tile([128, 512], F16, tag="qT", bufs=4)
                nc.scalar.copy(qT16[:], qT_ps[:])
                QA["kT"], QA["qT"] = kT16[:, 0:256], qT16[:, 0:256]
                QB["kT"], QB["qT"] = kT16[:, 256:512], qT16[:, 256:512]

                araw_ps = pscc.tile([128, 512], F32, tag="cc")
                for ci, Q in ((0, QA), (1, QB)):
                    for hh in range(2):
                        dsl = bass.ts(ci * 2 + hh, 128)
                        nc.tensor.matmul(araw_ps[:, dsl], Q["kT"][:, bass.ts(hh, 128)],
                                         Q["kT"][:, bass.ts(hh, 128)],
                                         start=True, stop=True)
                a16 = kpool.tile([128, 512], F16, tag="a16", bufs=4)
                for ci, Q in ((0, QA), (1, QB)):
                    for hh in range(2):
                        dsl = bass.ts(ci * 2 + hh, 128)
                        nc.vector.scalar_tensor_tensor(
                            a16[:, dsl], araw_ps[:, dsl],
                            Q["beta"][:, hh:hh + 1],
                            mtril_s[:], OP.mult, OP.mult)
                QA["a16"], QB["a16"] = a16[:, 0:256], a16[:, 256:512]

                at_ps = pscc.tile([128, 512], F16, tag="cc")
                for j in range(4):
                    nc.tensor.transpose(at_ps[:, bass.ts(j, 128)],
                                        a16[:, bass.ts(j, 128)], id_s[:])
                at16 = kpool.tile([128, 512], F16, tag="at16", bufs=4)
                nc.scalar.copy(at16[:], at_ps[:])
                x16 = npool.tile([128, 512], F16, tag="x16")
                nc.gpsimd.tensor_sub(x16[:], id4_s[:], a16[:])
                xt16 = npool.tile([128, 512], F16, tag="xt16", bufs=8)
                nc.gpsimd.tensor_sub(xt16[:], id4_s[:], at16[:])
                mt16 = kpool.tile([128, 512], F16, tag="mt16", bufs=4)
                nc.vector.tensor_add(mt16[:], id4_s[:], at16[:])

                # Newton (pair-batched plumbing; per-head matmuls)
                for it in range(NEWTON_ITERS):
                    t1_ps = pscc.tile([128, 512], F32, tag="cc")
                    for j in range(4):
                        jsl = bass.ts(j, 128)
                        nc.tensor.matmul(t1_ps[:, jsl], mt16[:, jsl],
                                         x16[:, jsl], start=True, stop=True)
                    t1n16 = npool.tile([128, 512], F16, tag="t1n")
                    nc.scalar.mul(t1n16[:], t1_ps[:], -1.0)
                    t2t_ps = pscc.tile([128, 512], F32, tag="cc")
                    for j in range(4):
                        jsl = bass.ts(j, 128)
                        nc.tensor.matmul(t2t_ps[:, jsl], t1n16[:, jsl],
                                         xt16[:, jsl], start=True, stop=True)
                    xt_new = npool.tile([128, 512], F16, tag="xt16", bufs=8)
                    nc.vector.scalar_tensor_tensor(
                        xt_new[:], xt16[:], 2.0, t2t_ps[:], OP.mult, OP.add)
                    if it < NEWTON_ITERS - 1:
                        t2_ps = pscc.tile([128, 512], F32, tag="cc")
                        for j in range(4):
                            jsl = bass.ts(j, 128)
                            nc.tensor.matmul(t2_ps[:, jsl], xt16[:, jsl],
                                             t1n16[:, jsl], start=True, stop=True)
                        x_new = npool.tile([128, 512], F16, tag="x16")
                        nc.vector.scalar_tensor_tensor(
                            x_new[:], x16[:], 2.0, t2_ps[:], OP.mult, OP.add)
                        x16 = x_new
                    xt16 = xt_new
                QA["xt"], QB["xt"] = xt16[:, 0:256], xt16[:, 256:512]

                wcn_ps = pscc.tile([128, 512], F32, tag="cc")
                for ci, Q in ((0, QA), (1, QB)):
                    for hh in range(2):
                        dsl = bass.ts(ci * 2 + hh, 128)
                        nc.tensor.matmul(wcn_ps[:, dsl],
                                         Q["kbn"][:, bass.ts(hh, 128)],
                                         Q["xt"][:, bass.ts(hh, 128)],
                                         start=True, stop=True)
                wcn16 = kpool.tile([128, 512], F16, tag="wcn", bufs=4)
                nc.scalar.copy(wcn16[:], wcn_ps[:])
                QA["wcn"], QB["wcn"] = wcn16[:, 0:256], wcn16[:, 256:512]
                graw_ps = pscc.tile([128, 512], F32, tag="cc")
                for ci, Q in ((0, QA), (1, QB)):
                    for hh in range(2):
                        dsl = bass.ts(ci * 2 + hh, 128)
                        nc.tensor.matmul(graw_ps[:, dsl],
                                         Q["kT"][:, bass.ts(hh, 128)],
                                         Q["qT"][:, bass.ts(hh, 128)],
                                         start=True, stop=True)
                gtm16 = kpool.tile([128, 512], F16, tag="gtm", bufs=4)
                nc.vector.tensor_mul(gtm16[:], graw_ps[:], mtriu4_s[:])
                QA["gtm"], QB["gtm"] = gtm16[:, 0:256], gtm16[:, 256:512]

            # ---- phase 2b: sequential scan per chunk ----
            for lc in range(n_cc_per_sc):
                Q = chunk_data[lc]
                kn16, kbn16 = Q["kn"], Q["kbn"]
                o16s = []
                for hh in range(2):
                    idx = lc * 2 + hh
                    hsl = bass.ts(hh, 128)
                    u_ps = psu.tile([128, DVH], F32, tag="u256")
                    nc.tensor.matmul(u_ps[:], Q["xt"][:, hsl],
                                     Q["bv"][:, hh * DVH:(hh + 1) * DVH],
                                     start=True, stop=False)
                    nc.tensor.matmul(u_ps[:], Q["wcn"][:, hsl], S16[hh][:],
                                     start=False, stop=True)
                    u16 = kpool.tile([128, DVH], F16, tag=f"u16_{hh}", bufs=6)
                    nc.scalar.copy(u16[:], u_ps[:])
                    o_ps = psu.tile([128, DVH], F32, tag="u256")
                    nc.tensor.matmul(o_ps[:], Q["qT"][:, hsl], S16[hh][:],
                                     start=True, stop=False)
                    nc.tensor.matmul(o_ps[:], Q["gtm"][:, hsl], u16[:],
                                     start=False, stop=True)
                    sd_ps = psu.tile([128, DVH], F32, tag="u256")
                    nc.tensor.matmul(sd_ps[:], kn16[:, hsl], u16[:],
                                     start=True, stop=True)
                    # fp16 working state directly (keeps chain short);
                    # fp32 master updated off the critical path
                    s16_new = spool.tile([128, DVH], F16, tag=f"S16_{hh}")
                    nc.vector.scalar_tensor_tensor(
                        s16_new[:], S32[hh][:], 1.0, sd_ps[:], OP.mult, OP.add)
                    s32_new = spool.tile([128, DVH], F32, tag=f"S32_{hh}")
                    nc.vector.tensor_add(s32_new[:], S32[hh][:], sd_ps[:])
                    S32[hh] = s32_new
                    S16[hh] = s16_new
                    o16 = kpool.tile([128, DVH], F16, tag=f"o16_{hh}", bufs=6)
                    nc.scalar.copy(o16[:], o_ps[:])
                    scrv = kpool.tile([128, DVH], F16, tag="scrv")
                    nc.scalar.activation(scrv[:], o16[:], AF.Square,
                                         accum_out=ms_sc[:, idx:idx + 1])
                    o16s.append(o16)
                Q["o16"] = o16s

            # z = sum(o^2) + DVH*eps*||q||^2
            nc.vector.scalar_tensor_tensor(
                z_sc[:], sqq_sc[:], float(DVH * NORM_EPS), ms_sc[:],
                OP.mult, OP.add)
            rz = rsqrt_dve(z_sc[:], 8, "rsc")
            rsc_sc = kpool.tile([128, 8], F32, tag="rsc_sc")
            nc.vector.tensor_scalar_mul(rsc_sc[:], rz, float(DVH ** 0.5))

            # ---- phase 3: gate + output projection (grouped) ----
            for lc in range(n_cc_per_sc):
                Q = chunk_data[lc]
                og16 = kpool.tile([128, 512], F16, tag="og", bufs=5)
                for hh in range(2):
                    idx = lc * 2 + hh
                    nc.vector.scalar_tensor_tensor(
                        og16[:, hh * DVH:(hh + 1) * DVH], Q["o16"][hh][:],
                        rsc_sc[:, idx:idx + 1],
                        Q["sg"][:, hh * DVH:(hh + 1) * DVH],
                        OP.mult, OP.mult)
                Q["og"] = og16
            for lc in range(n_cc_per_sc):
                Q = chunk_data[lc]
                ogt16 = kpool.tile([128, 512], F16, tag="ogt", bufs=5)
                for jp in range(2):
                    ogt_ps = pscc.tile([128, 256], F16, tag="cc")
                    for j2 in range(2):
                        j = jp * 2 + j2
                        nc.tensor.transpose(ogt_ps[:, bass.ts(j2, 128)],
                                            Q["og"][:, bass.ts(j, 128)],
                                            id_s[:])
                    nc.vector.tensor_copy(ogt16[:, bass.ts(jp, 256)], ogt_ps[:])
                Q["ogt"] = ogt16
            for lc in range(n_cc_per_sc):
                cc = sc * n_cc_per_sc + lc
                Q = chunk_data[lc]
                ogt16 = Q["ogt"]
                out32 = opool.tile([128, D], F32, tag="out32")
                for grp in range(2):
                    op_ps = psop.tile([128, 512], F32, tag="op512")
                    for j in range(4):
                        nc.tensor.matmul(op_ps[:], ogt16[:, bass.ts(j, 128)],
                                         wo_s[:, j, bass.ts(grp, 512)],
                                         start=(j == 0), stop=(j == 3))
                    if grp == 0:
                        nc.vector.tensor_copy(out32[:, bass.ts(grp, 512)],
                                              op_ps[:])
                    else:
                        nc.scalar.copy(out32[:, bass.ts(grp, 512)], op_ps[:])
                nc.sync.dma_start(out_d[cc * C:(cc + 1) * C, :], out32[:])

    return nc



def _split_drain_waits(nc):
    """Walrus enforces small per-instruction sync-wait capacities (1 for
    Drain, 2 observed-safe elsewhere). Hoist overflow waits onto preceding
    same-engine Drain instructions (1 wait each)."""
    import copy
    import concourse.mybir as mybir
    for f in nc.m.functions:
        for bb in f.blocks:
            new_insts = []
            for inst in bb.instructions:
                si = inst.sync_info
                limit = 1
                if si is not None and si.on_wait and len(si.on_wait) > limit:
                    waits = list(si.on_wait)
                    keep = waits[-limit:]
                    hoist = waits[:-limit]
                    for ci, w in enumerate(hoist):
                        d = mybir.InstDrain(
                            name=f"{inst.name}-ws{ci}",
                            ins=[], outs=[],
                            sync_info=mybir.SyncInfo(on_wait=[w], on_update=[]),
                        )
                        d.engine = inst.engine
                        new_insts.append(d)
                    inst.sync_info.on_wait = keep
                new_insts.append(inst)
            bb.instructions[:] = new_insts


def _get_built():
    global _BUILT
    if _BUILT is None:
        _BUILT = _build()
        _split_drain_waits(_BUILT)
    return _BUILT


def _prep_core_inputs(c, inputs):
    """Host-side sharding/layout prep for core c."""
    f16 = np.float16
    b = c // 2
    p = c % 2
    hs = [2 * p, 2 * p + 1]

    x = np.asarray(inputs["hidden_states"], np.float32)[b]        # [L, D]
    xT = np.zeros((D, L + KC - 1), np.float16)
    xT[:, KC - 1:] = x.T.astype(f16)

    Wq = np.asarray(inputs["Wq"], np.float32)
    Wk = np.asarray(inputs["Wk"], np.float32)
    Wv = np.asarray(inputs["Wv"], np.float32)
    Wb = np.asarray(inputs["Wb"], np.float32)
    Wg = np.asarray(inputs["Wg"], np.float32)
    Wo = np.asarray(inputs["Wo"], np.float32)
    norm_w = np.asarray(inputs["norm_w"], np.float32)
    conv_w = np.asarray(inputs["conv_w"], np.float32)

    qcols = np.concatenate([Wq[:, h * DQH:(h + 1) * DQH] for h in hs], axis=1)
    kcols = np.concatenate([Wk[:, h * DQH:(h + 1) * DQH] for h in hs], axis=1)
    wqk = np.concatenate([qcols, kcols], axis=1).astype(f16)       # [D, 512]
    wv = np.concatenate([Wv[:, h * DVH:(h + 1) * DVH] for h in hs],
                        axis=1).astype(f16)
    wg = np.concatenate([Wg[:, h * DVH:(h + 1) * DVH] for h in hs],
                        axis=1).astype(f16)
    wb = Wb[:, hs].astype(f16)
    Wo_eff = Wo * np.tile(norm_w, H)[:, None]
    wo = Wo_eff[p * 512:(p + 1) * 512, :].astype(f16)

    return {
        "xT": xT,
        "wqk": wqk, "wv": wv, "wg": wg, "wb": wb, "wo": wo,
        "cw": conv_w.astype(np.float32),
        "cwdiag": np.concatenate(
            [np.diag(conv_w[k * 128:(k + 1) * 128, i]).astype(np.float16)
             for k in range(D // 128) for i in range(KC)], axis=0),
        "ident": np.eye(128, dtype=np.float16),
        "ident4": np.concatenate([np.eye(128, dtype=np.float16)] * 4, axis=1),
        "mtril": np.tril(np.ones((128, 128), np.float32), -1),
        "mtriu4": np.concatenate(
            [np.triu(np.ones((128, 128), np.float32), 0)] * 4, axis=1),
    }


def kernel(**inputs):
    from concourse.bass_utils import run_bass_kernel_spmd

    nc = _get_built()
    in_maps = [_prep_core_inputs(c, inputs) for c in range(N_CORES)]
    res = run_bass_kernel_spmd(nc, in_maps, core_ids=list(range(N_CORES)),
                               trace=TRACE, **TRACE_KW)
    kernel.last_results = res
    out = np.zeros((B, L, D), np.float32)
    for b in range(B):
        out[b] = res.results[2 * b]["out"] + res.results[2 * b + 1]["out"]
    return out
